# revision 5
# baseline (speedup 1.0000x reference)
"""Trainium2 kernel for the modality-softmax attention problem.

    scores  = tanh(einsum("mbd,ed->mbe", x, W))
    weights = softmax(scores, axis=0)            # over M modalities
    out     = sum_m x * weights                  # [B, D]
    out    *= (1 + #modalities whose feature-sum is exactly 0)[b]

Sharding: data-parallel over the batch dim — 8 NeuronCores x B/8 rows,
W replicated. Everything on-chip runs transposed ([feature, batch]).

The score matmuls (the FLOP bottleneck: 2*M*B*D^2 = 412 GFLOP) run in
fp8e4 with perf_mode=DoubleRow: each matmul contracts K=256 (two 128-row
sub-chunks packed per PE cell) in the same ~N cycles a bf16 K=128 matmul
takes — a true 2x (measured 216 ns per K=256,N=512 matmul, DR ldweights
fully hidden). W is scaled by 16 before fp8 quantization (its entries
~N(0, 1/D) would land in the subnormal range) and the 1/16 is folded into
the tanh activation's scale. The elementwise path runs in fp16 to keep
quantization error concentrated in the fp8 matmul: host ships two copies
of x — fp8 for the matmul moving operand, fp16 for the elementwise
operand (d-major layout serves both roles since D == E).

Engine budget per core (b=live batch columns): Tensor ~311 us is the
roofline. ACT: tanh+exp (~250 us). DVE can't double-pump tensor_tensor
on trn2 (~415 ns per 512-col fp16 op), so the elementwise work is split:
DVE does the x*e product + numerator add, GpSimd does the denominator
add and the finalize scaler fold. DMA issue rings: x8+wt+out on sync,
xe on gpsimd, keeping the ACT queue free for activations. Each
modality's x tiles are DMA-issued one modality ahead (the Tile
framework's WAR semaphores make the transfer start exactly when the
buffer frees), hiding the load under the previous modality's matmuls.

tanh scores lie in [-1,1], so softmax is computed without max-subtraction
as (sum x*exp(tanh s)) / (sum exp(tanh s)).

Missing-modality rows (all-zero x[m, b, :]) contribute exp(0)=1 to the
softmax denominator and 0 to the numerator, so their matmuls are pure
waste. The host detects them, permutes the batch so each core sees the
same per-modality all-zero prefix, and the kernel is built with those
prefixes statically skipped (shorter matmul N, Den bumped by a constant).
The permutation is undone on the host after the gather. With no zero rows
the plan degenerates to a dense kernel.

Zero-row detection for the rescale: a modality column is all-zero iff its
scores are exactly 0 (fp accumulate of zeros is exact; fp8(0)=0), checked
on the first ZDET_CHUNKS e-chunks of tanh(s); the flags land replicated
across partitions so no cross-partition traffic is needed. The per-ec
finalize (1/Den via fast Newton reciprocal, muls, store) is emitted inside
the last modality's loop so it overlaps the remaining matmuls.
"""

from contextlib import ExitStack

import numpy as np
import ml_dtypes

import concourse.bass as bass
import concourse.bacc as bacc
import concourse.mybir as mybir
import concourse.tile as tile
from concourse.bass_utils import run_bass_kernel_spmd

F32 = mybir.dt.float32
FP16 = mybir.dt.float16
FP8 = mybir.dt.float8e4
P = 128
N_CORES = 8
ZDET_CHUNKS = 2
W_SCALE = 16.0  # pow2: exact in fp; folded into tanh via activation scale


def build_kernel(M, D, E, Bc, BT, skips=None):
    """Build the per-core Bass graph.

    M: modalities, D: contraction dim, E: output feature dim, Bc: per-core
    batch, BT: batch tile (matmul N). skips[m] = per-core all-zero prefix
    length for modality m (columns statically skipped).
    """
    DC = D // P
    D2 = DC // 2  # DoubleRow chunks of K=256
    EC = E // P
    NBT = Bc // BT
    assert D % (2 * P) == 0 and E % P == 0 and Bc % BT == 0
    skips = list(skips or [0] * M)
    assert len(skips) == M and all(0 <= k <= Bc for k in skips)

    nc = bacc.Bacc()

    x8 = nc.declare_dram_parameter("x8", [M, DC, P, Bc], FP8, isOutput=False)
    xe = nc.declare_dram_parameter("xe", [M, DC, P, Bc], FP16, isOutput=False)
    wt = nc.declare_dram_parameter("wt", [EC, DC, P, P], FP8, isOutput=False)
    outT = nc.declare_dram_parameter("outT", [E, Bc], F32, isOutput=True)

    with tile.TileContext(nc) as tc, ExitStack() as ctx:
        singles = ctx.enter_context(tc.tile_pool(name="singles", bufs=1))
        xe_pool = ctx.enter_context(tc.tile_pool(name="xe", bufs=2))
        x8_pool = ctx.enter_context(tc.tile_pool(name="x8", bufs=2))
        acc_pool = ctx.enter_context(tc.tile_pool(name="acc", bufs=1))
        e_pool = ctx.enter_context(tc.tile_pool(name="e", bufs=4))
        t_pool = ctx.enter_context(tc.tile_pool(name="t", bufs=3))
        prod_pool = ctx.enter_context(tc.tile_pool(name="prod", bufs=3))
        scaler_pool = ctx.enter_context(tc.tile_pool(name="scaler", bufs=2))
        z_pool = ctx.enter_context(tc.tile_pool(name="z", bufs=2))
        out_pool = ctx.enter_context(tc.tile_pool(name="out", bufs=3))
        rec_pool = ctx.enter_context(tc.tile_pool(name="rec", bufs=3))
        sc_psum = ctx.enter_context(tc.tile_pool(name="scps", bufs=4, space="PSUM"))

        dram_pool = ctx.enter_context(tc.tile_pool(name="wudram", bufs=1, space="DRAM"))
        wu_sb = singles.tile([P, 64], mybir.dt.bfloat16)
        nc.vector.memset(wu_sb, 1.0)
        wu_ps = sc_psum.tile([P, 64], F32)
        for i in range(64):
            nc.tensor.matmul(
                wu_ps[:64], lhsT=wu_sb[:, :64], rhs=wu_sb,
                start=(i == 0), stop=(i == 63),
            )
        wu_out = singles.tile([P, 64], F32)
        nc.vector.tensor_copy(wu_out[:64], wu_ps[:64])
        wu_dram = dram_pool.tile([P, 64], F32)
        nc.sync.dma_start(out=wu_dram[:64], in_=wu_out[:64])

        # Replicated fp8 weight, resident for the whole kernel. e-chunk-major
        # DMAs: the first score group (ec=0) only waits for 1/16 of the load.
        wt_sb = singles.tile([P, DC, E], FP8)
        for ec in range(EC):
            if ec == 0:
                for q in range(0, DC, max(DC // 4, 1)):
                    qe = min(q + max(DC // 4, 1), DC)
                    nc.sync.dma_start(
                        out=wt_sb[:, q:qe, :P],
                        in_=wt[0, q:qe].rearrange("dc p j -> p dc j"),
                    )
            else:
                nc.sync.dma_start(
                    out=wt_sb[:, :, ec * P : (ec + 1) * P],
                    in_=wt[ec].rearrange("dc p j -> p dc j"),
                )

        # Work items: one per live (b-tile, modality); x-tile DMAs are issued
        # one item ahead of compute.
        items = []
        for bt in range(NBT):
            for m in range(M):
                sk_m = min(max(skips[m] - bt * BT, 0), BT)
                if sk_m < BT:
                    items.append((bt, m, sk_m))

        tiles = {}

        def issue_loads(it):
            bt, m, sk_m = it
            lo = sk_m
            bs = bt * BT
            xe_t = xe_pool.tile([P, DC, BT], FP16)
            x8_t = x8_pool.tile([P, DC, BT], FP8)
            for q in range(0, DC, max(DC // 4, 1)):
                qe = min(q + max(DC // 4, 1), DC)
                nc.sync.dma_start(
                    out=x8_t[:, q:qe, lo:],
                    in_=x8[m, q:qe, :, bs + lo : bs + BT].rearrange("dc p b -> p dc b"),
                )
                nc.gpsimd.dma_start(
                    out=xe_t[:, q:qe, lo:],
                    in_=xe[m, q:qe, :, bs + lo : bs + BT].rearrange("dc p b -> p dc b"),
                )
            tiles[it] = (xe_t, x8_t)

        issue_loads(items[0])

        cur_bt = -1
        scaler_acc = n_sb = d_sb = None
        sk = wm = None
        w_final = 0

        for idx, it in enumerate(items):
            bt, m, _ = it
            if idx + 1 < len(items):
                issue_loads(items[idx + 1])

            if bt != cur_bt:
                cur_bt = bt
                sk = [min(max(skips[mm] - bt * BT, 0), BT) for mm in range(M)]
                wm = [BT] * (M + 1)
                for mm in range(M):
                    wm[mm + 1] = min(wm[mm], sk[mm])
                w_final = wm[M]
                # scaler_acc[e, b] = 1 + #m with all-zero column b (replicated
                # over e). Skipped prefixes are added statically; live ranges
                # come from the z_m flags.
                scaler_acc = scaler_pool.tile([P, BT], F32)
                nc.vector.memset(scaler_acc, 1.0)
                for mm in range(M):
                    if sk[mm] > 0:
                        nc.vector.tensor_scalar_add(
                            scaler_acc[:, : sk[mm]], scaler_acc[:, : sk[mm]], 1.0
                        )
                n_sb = acc_pool.tile([P, EC, BT], FP16, tag="num")
                d_sb = acc_pool.tile([P, EC, BT], F32, tag="den")

            def finalize(ec, scaler_acc=scaler_acc, n_sb=n_sb, d_sb=d_sb,
                         bt=bt, sk=sk, w_final=w_final, split=1):
                # statically-skipped modalities contribute exp(0)=1 to Den
                if w_final > 0:
                    nc.vector.memset(n_sb[:, ec, :w_final], 0.0)
                    nc.vector.memset(d_sb[:, ec, :w_final], 0.0)
                for mm in range(M):
                    if sk[mm] > 0:
                        nc.gpsimd.tensor_scalar_add(
                            d_sb[:, ec, : sk[mm]], d_sb[:, ec, : sk[mm]], 1.0
                        )
                H = BT // split
                for h in range(split):
                    hs = slice(h * H, (h + 1) * H)
                    r_t = rec_pool.tile([P, BT], F32, tag="recip")
                    # Den in [1, M*e]: no edge cases; 51-ULP approx is plenty
                    nc.vector.reciprocal_approx_fast(
                        out=r_t[:, hs], in_=d_sb[:, ec, hs]
                    )
                    # fold the missing-modality scaler into the reciprocal
                    nc.gpsimd.tensor_mul(r_t[:, hs], r_t[:, hs], scaler_acc[:, hs])
                    o_t = out_pool.tile([P, BT], F32)
                    nc.vector.tensor_mul(o_t[:, hs], n_sb[:, ec, hs], r_t[:, hs])
                    nc.sync.dma_start(
                        out=outT[ec * P : (ec + 1) * P, bt * BT + h * H : bt * BT + (h + 1) * H],
                        in_=o_t[:, hs],
                    )

            lo = sk[m]  # live range [lo, BT)
            ini_hi = wm[m]  # init range [lo, ini_hi), accum [ini_hi, BT)
            xe_t, x8_t = tiles.pop(it)
            z_m = z_pool.tile([P, BT], F32)
            last_m = m == M - 1

            for ec in range(EC):
                sc_ps = sc_psum.tile([P, BT], F32)
                for d2 in range(D2):
                    nc.tensor.matmul(
                        sc_ps[:, lo:],
                        lhsT=wt_sb[:, 2 * d2 : 2 * d2 + 2, ec * P : (ec + 1) * P],
                        rhs=x8_t[:, 2 * d2 : 2 * d2 + 2, lo:],
                        start=(d2 == 0),
                        stop=(d2 == D2 - 1),
                        perf_mode=mybir.MatmulPerfMode.DoubleRow,
                    )
                t_t = t_pool.tile([P, BT], F32, tag="tanh")
                nc.scalar.activation(
                    t_t[:, lo:], sc_ps[:, lo:],
                    mybir.ActivationFunctionType.Tanh,
                    scale=1.0 / W_SCALE,
                )
                # exp: init region writes Den directly, accum region to e_t
                e_t = None
                if ini_hi > lo:
                    nc.scalar.activation(
                        d_sb[:, ec, lo:ini_hi],
                        t_t[:, lo:ini_hi],
                        mybir.ActivationFunctionType.Exp,
                    )
                if BT > ini_hi:
                    e_t = e_pool.tile([P, BT], FP16, tag="exp")
                    nc.scalar.activation(
                        e_t[:, ini_hi:],
                        t_t[:, ini_hi:],
                        mybir.ActivationFunctionType.Exp,
                    )

                # z_m = prod_ec<ZDET (tanh(s) == 0) over the live range
                if ec == 0:
                    nc.vector.tensor_single_scalar(
                        out=z_m[:, lo:],
                        in_=t_t[:, lo:],
                        scalar=0.0,
                        op=mybir.AluOpType.is_equal,
                    )
                elif ec < ZDET_CHUNKS:
                    nc.vector.scalar_tensor_tensor(
                        out=z_m[:, lo:],
                        in0=t_t[:, lo:],
                        scalar=0.0,
                        in1=z_m[:, lo:],
                        op0=mybir.AluOpType.is_equal,
                        op1=mybir.AluOpType.mult,
                    )
                    if ec == ZDET_CHUNKS - 1:
                        nc.vector.tensor_add(
                            scaler_acc[:, lo:], scaler_acc[:, lo:], z_m[:, lo:]
                        )

                if ini_hi > lo:
                    nc.vector.tensor_mul(
                        n_sb[:, ec, lo:ini_hi],
                        xe_t[:, ec, lo:ini_hi],
                        d_sb[:, ec, lo:ini_hi],
                    )
                if BT > ini_hi:
                    p_t = prod_pool.tile([P, BT], FP16, tag="prod")
                    nc.vector.tensor_mul(
                        p_t[:, ini_hi:], xe_t[:, ec, ini_hi:], e_t[:, ini_hi:]
                    )
                    nc.vector.tensor_add(
                        n_sb[:, ec, ini_hi:], n_sb[:, ec, ini_hi:], p_t[:, ini_hi:]
                    )
                    nc.gpsimd.tensor_add(
                        d_sb[:, ec, ini_hi:], d_sb[:, ec, ini_hi:], e_t[:, ini_hi:]
                    )

                if last_m and ec >= ZDET_CHUNKS - 1:
                    # scaler is ready; finalize chunks as they complete
                    last_chain = bt == NBT - 1 and ec == EC - 1
                    if ec == ZDET_CHUNKS - 1:
                        for past_ec in range(ZDET_CHUNKS):
                            finalize(past_ec)
                    else:
                        finalize(ec, split=4 if last_chain else 1)

    nc.compile()
    return nc


def plan_shards(x):
    """Assign batch rows to cores so every core sees the same per-modality
    all-zero prefix. Returns (perm [n_cores, Bc] of global row ids,
    skips [M] prefix lengths)."""
    M, B, D = x.shape
    Bc = B // N_CORES
    zero = np.all(x == 0.0, axis=2)  # [M, B] truly-all-zero rows
    zcount = zero.sum(axis=0)
    sig = np.zeros(B, dtype=np.int64)
    for m in range(M):
        sig |= zero[m].astype(np.int64) << m
    # cluster by signature with most-zero rows first; round-robin over cores
    order = np.lexsort((sig, -zcount))
    perm = np.stack([order[c::N_CORES] for c in range(N_CORES)])
    skips = []
    for m in range(M):
        k = Bc
        for c in range(N_CORES):
            nz = np.flatnonzero(~zero[m][perm[c]])
            k = min(k, int(nz[0]) if len(nz) else Bc)
        skips.append(k)
    return perm, skips


def prepare_inputs(x, W, perm):
    """Host-side packing: per-core permuted shard, transposed to [d, b];
    fp8 copy for the matmul path, fp16 copy for the elementwise path."""
    M, B, D = x.shape
    Bc = B // N_CORES
    DC = D // P
    EC = D // P
    wt = np.ascontiguousarray(
        (W.T * W_SCALE)
        .astype(ml_dtypes.float8_e4m3)
        .reshape(DC, P, EC, P)
        .transpose(2, 0, 1, 3)
    )
    x8b = x.astype(ml_dtypes.float8_e4m3)
    xeb = x.astype(np.float16)
    in_maps = []
    for c in range(N_CORES):
        x8c = np.ascontiguousarray(
            x8b[:, perm[c], :].transpose(0, 2, 1)
        ).reshape(M, DC, P, Bc)
        xec = np.ascontiguousarray(
            xeb[:, perm[c], :].transpose(0, 2, 1)
        ).reshape(M, DC, P, Bc)
        in_maps.append({"x8": x8c, "xe": xec, "wt": wt})
    return in_maps


_NC_CACHE = {}


def kernel(x, W, _trace=False, **trace_kwargs):
    x = np.asarray(x)
    W = np.asarray(W)
    M, B, D = x.shape
    Bc = B // N_CORES
    BT = 512 if Bc % 512 == 0 else Bc
    perm, skips = plan_shards(x)
    key = (M, B, D, tuple(skips))
    if key not in _NC_CACHE:
        _NC_CACHE[key] = build_kernel(M, D, D, Bc, BT, skips=skips)
    nc = _NC_CACHE[key]
    in_maps = prepare_inputs(x, W, perm)
    res = run_bass_kernel_spmd(
        nc, in_maps, core_ids=list(range(N_CORES)), trace=_trace, **trace_kwargs
    )
    out = np.empty((B, D), np.float32)
    for c in range(N_CORES):
        out[perm[c], :] = res.results[c]["outT"].T
    if _trace:
        return out, res
    return out


# revision 7
# speedup vs baseline: 1.2857x; 1.2857x over previous
"""Trainium2 kernel for the modality-softmax attention problem.

    scores  = tanh(einsum("mbd,ed->mbe", x, W))
    weights = softmax(scores, axis=0)            # over M modalities
    out     = sum_m x * weights                  # [B, D]
    out    *= (1 + #modalities whose feature-sum is exactly 0)[b]

Sharding: data-parallel over the batch dim — 8 NeuronCores x B/8 rows,
W replicated. Everything on-chip runs transposed ([feature, batch]).

The score matmuls (the FLOP bottleneck: 2*M*B*D^2 = 412 GFLOP) run in
fp8e4 with perf_mode=DoubleRow: each matmul contracts K=256 (two 128-row
sub-chunks packed per PE cell) in the same ~N cycles a bf16 K=128 matmul
takes — a true 2x (measured 216 ns per K=256,N=512 matmul, DR ldweights
fully hidden). W is scaled by 16 before fp8 quantization (its entries
~N(0, 1/D) would land in the subnormal range) and the 1/16 is folded into
the tanh activation's scale. The elementwise path runs in fp16 to keep
quantization error concentrated in the fp8 matmul: host ships two copies
of x — fp8 for the matmul moving operand, fp16 for the elementwise
operand (d-major layout serves both roles since D == E).

Engine budget per core: Tensor ~311 us is the roofline. ACT tanh+exp
~250 us. DVE tensor_tensor is capped at 2x_1p (16-bit) / 1x (fp32) with
a fixed per-op overhead, so the softmax accumulation processes TWO
128-row e-chunks per DVE op ([P, 2, BT] APs) to amortize it; GpSimd
(slow Q7 DSP, ~1.5 us per elementwise op) only issues the xe DMAs.
DMA rings: x8+wt+out on sync, xe on gpsimd, keeping the ACT queue free
for activations. Each modality's x tiles are DMA-issued one work-item
ahead; the Tile framework's WAR semaphores start the transfer exactly
when the previous buffer frees, hiding the load under compute.

tanh scores lie in [-1,1], so softmax is computed without max-subtraction
as (sum x*exp(tanh s)) / (sum exp(tanh s)).

Missing-modality rows (all-zero x[m, b, :]) contribute exp(0)=1 to the
softmax denominator and 0 to the numerator, so their matmuls are pure
waste. The host detects them, permutes the batch so each core sees the
same per-modality all-zero prefix, and the kernel is built with those
prefixes statically skipped (shorter matmul N, Den bumped by a constant).
The permutation is undone on the host after the gather. With no zero rows
the plan degenerates to a dense kernel.

Zero-row detection for the rescale: a modality column is all-zero iff its
scores are exactly 0 (fp accumulate of zeros is exact; fp8(0)=0), checked
on the first ZDET_CHUNKS e-chunks of tanh(s); the flags land replicated
across partitions so no cross-partition traffic is needed. The per-pair
finalize (1/Den via fast Newton reciprocal, muls, store) is emitted inside
the last modality's loop so it overlaps the remaining matmuls.
"""

from contextlib import ExitStack

import numpy as np
import ml_dtypes

import concourse.bass as bass
import concourse.bacc as bacc
import concourse.mybir as mybir
import concourse.tile as tile
from concourse.bass_utils import run_bass_kernel_spmd

F32 = mybir.dt.float32
FP16 = mybir.dt.float16
FP8 = mybir.dt.float8e4
P = 128
N_CORES = 8
ZDET_CHUNKS = 2
W_SCALE = 16.0  # pow2: exact in fp; folded into tanh via activation scale


def build_kernel(M, D, E, Bc, BT, skips=None):
    """Build the per-core Bass graph.

    M: modalities, D: contraction dim, E: output feature dim, Bc: per-core
    batch, BT: batch tile (matmul N). skips[m] = per-core all-zero prefix
    length for modality m (columns statically skipped).
    """
    DC = D // P
    D2 = DC // 2  # DoubleRow chunks of K=256
    EC = E // P
    EP = EC // 2  # e-chunk pairs for DVE op fusion
    NBT = Bc // BT
    assert D % (2 * P) == 0 and E % (2 * P) == 0 and Bc % BT == 0
    assert ZDET_CHUNKS == 2, "zdet assumed to complete within the first pair"
    skips = list(skips or [0] * M)
    assert len(skips) == M and all(0 <= k <= Bc for k in skips)

    nc = bacc.Bacc()

    x8 = nc.declare_dram_parameter("x8", [M, DC, P, Bc], FP8, isOutput=False)
    xe = nc.declare_dram_parameter("xe", [M, DC, P, Bc], FP16, isOutput=False)
    wt = nc.declare_dram_parameter("wt", [EC, DC, P, P], FP8, isOutput=False)
    outT = nc.declare_dram_parameter("outT", [E, Bc], F32, isOutput=True)

    with tile.TileContext(nc) as tc, ExitStack() as ctx:
        singles = ctx.enter_context(tc.tile_pool(name="singles", bufs=1))
        xe_pool = ctx.enter_context(tc.tile_pool(name="xe", bufs=2))
        x8_pool = ctx.enter_context(tc.tile_pool(name="x8", bufs=2))
        acc_pool = ctx.enter_context(tc.tile_pool(name="acc", bufs=1))
        e_pool = ctx.enter_context(tc.tile_pool(name="e", bufs=3))
        t_pool = ctx.enter_context(tc.tile_pool(name="t", bufs=3))
        prod_pool = ctx.enter_context(tc.tile_pool(name="prod", bufs=2))
        scaler_pool = ctx.enter_context(tc.tile_pool(name="scaler", bufs=2))
        z_pool = ctx.enter_context(tc.tile_pool(name="z", bufs=2))
        out_pool = ctx.enter_context(tc.tile_pool(name="out", bufs=3))
        rec_pool = ctx.enter_context(tc.tile_pool(name="rec", bufs=3))
        sc_psum = ctx.enter_context(tc.tile_pool(name="scps", bufs=4, space="PSUM"))

        dram_pool = ctx.enter_context(tc.tile_pool(name="wudram", bufs=1, space="DRAM"))
        wu_sb = singles.tile([P, 64], mybir.dt.bfloat16)
        nc.vector.memset(wu_sb, 1.0)
        wu_ps = sc_psum.tile([P, 64], F32)
        for i in range(64):
            nc.tensor.matmul(
                wu_ps[:64], lhsT=wu_sb[:, :64], rhs=wu_sb,
                start=(i == 0), stop=(i == 63),
            )
        wu_out = singles.tile([P, 64], F32)
        nc.vector.tensor_copy(wu_out[:64], wu_ps[:64])
        wu_dram = dram_pool.tile([P, 64], F32)
        nc.sync.dma_start(out=wu_dram[:64], in_=wu_out[:64])

        # Replicated fp8 weight, resident for the whole kernel. e-chunk-major
        # DMAs: the first score group (ec=0) only waits for 1/16 of the load.
        wt_sb = singles.tile([P, DC, E], FP8)
        for ec in range(EC):
            if ec == 0:
                for q in range(0, DC, max(DC // 4, 1)):
                    qe = min(q + max(DC // 4, 1), DC)
                    nc.sync.dma_start(
                        out=wt_sb[:, q:qe, :P],
                        in_=wt[0, q:qe].rearrange("dc p j -> p dc j"),
                    )
            else:
                nc.sync.dma_start(
                    out=wt_sb[:, :, ec * P : (ec + 1) * P],
                    in_=wt[ec].rearrange("dc p j -> p dc j"),
                )

        # Work items: one per live (b-tile, modality); x-tile DMAs are issued
        # one item ahead of compute.
        items = []
        for bt in range(NBT):
            for m in range(M):
                sk_m = min(max(skips[m] - bt * BT, 0), BT)
                if sk_m < BT:
                    items.append((bt, m, sk_m))

        tiles = {}

        def issue_loads(it):
            bt, m, sk_m = it
            lo = sk_m
            bs = bt * BT
            xe_t = xe_pool.tile([P, DC, BT], FP16)
            x8_t = x8_pool.tile([P, DC, BT], FP8)
            for q in range(0, DC, max(DC // 4, 1)):
                qe = min(q + max(DC // 4, 1), DC)
                nc.sync.dma_start(
                    out=x8_t[:, q:qe, lo:],
                    in_=x8[m, q:qe, :, bs + lo : bs + BT].rearrange("dc p b -> p dc b"),
                )
                nc.gpsimd.dma_start(
                    out=xe_t[:, q:qe, lo:],
                    in_=xe[m, q:qe, :, bs + lo : bs + BT].rearrange("dc p b -> p dc b"),
                )
            tiles[it] = (xe_t, x8_t)

        issue_loads(items[0])

        cur_bt = -1
        scaler_acc = scaler2 = n_sb = d_sb = None
        sk = wm = None
        w_final = 0

        for idx, it in enumerate(items):
            bt, m, _ = it
            if idx + 1 < len(items):
                issue_loads(items[idx + 1])

            if bt != cur_bt:
                cur_bt = bt
                sk = [min(max(skips[mm] - bt * BT, 0), BT) for mm in range(M)]
                wm = [BT] * (M + 1)
                for mm in range(M):
                    wm[mm + 1] = min(wm[mm], sk[mm])
                w_final = wm[M]
                # scaler_acc[e, b] = 1 + #m with all-zero column b (replicated
                # over e). Skipped prefixes are added statically; live ranges
                # come from the z_m flags.
                scaler_acc = scaler_pool.tile([P, BT], F32)
                nc.vector.memset(scaler_acc, 1.0)
                for mm in range(M):
                    if sk[mm] > 0:
                        nc.vector.tensor_scalar_add(
                            scaler_acc[:, : sk[mm]], scaler_acc[:, : sk[mm]], 1.0
                        )
                # duplicated over the pair dim for the pair-wide finalize mul
                scaler2 = scaler_pool.tile([P, 2, BT], F32, tag="s2")
                n_sb = acc_pool.tile([P, EC, BT], FP16, tag="num")
                d_sb = acc_pool.tile([P, EC, BT], F32, tag="den")

            def finalize(ep, scaler2=scaler2, n_sb=n_sb, d_sb=d_sb,
                         bt=bt, sk=sk, w_final=w_final, split=1):
                ec0 = 2 * ep
                ecs = slice(ec0, ec0 + 2)
                # statically-skipped modalities contribute exp(0)=1 to Den
                if w_final > 0:
                    nc.vector.memset(n_sb[:, ecs, :w_final], 0.0)
                    nc.vector.memset(d_sb[:, ecs, :w_final], 0.0)
                for mm in range(M):
                    if sk[mm] > 0:
                        nc.vector.tensor_scalar_add(
                            d_sb[:, ecs, : sk[mm]], d_sb[:, ecs, : sk[mm]], 1.0
                        )
                H = BT // split
                for h in range(split):
                    hs = slice(h * H, (h + 1) * H)
                    r_t = rec_pool.tile([P, 2, BT], F32, tag="recip")
                    # Den in [1, M*e]: no edge cases; 51-ULP approx is plenty
                    nc.vector.reciprocal_approx_fast(
                        out=r_t[:, :, hs], in_=d_sb[:, ecs, hs]
                    )
                    # fold the missing-modality scaler into the reciprocal
                    nc.vector.tensor_mul(r_t[:, :, hs], r_t[:, :, hs], scaler2[:, :, hs])
                    o_t = out_pool.tile([P, 2, BT], F32)
                    nc.vector.tensor_mul(o_t[:, :, hs], n_sb[:, ecs, hs], r_t[:, :, hs])
                    nc.sync.dma_start(
                        out=outT[
                            ec0 * P : (ec0 + 2) * P,
                            bt * BT + h * H : bt * BT + (h + 1) * H,
                        ].rearrange("(c p) b -> p c b", c=2),
                        in_=o_t[:, :, hs],
                    )

            lo = sk[m]  # live range [lo, BT)
            ini_hi = wm[m]  # init range [lo, ini_hi), accum [ini_hi, BT)
            xe_t, x8_t = tiles.pop(it)
            z_m = z_pool.tile([P, BT], F32)
            last_m = m == M - 1

            for ep in range(EP):
                e2 = None
                if BT > ini_hi:
                    e2 = e_pool.tile([P, 2, BT], FP16, tag="exp")
                for j in range(2):
                    ec = 2 * ep + j
                    sc_ps = sc_psum.tile([P, BT], F32)
                    for d2 in range(D2):
                        nc.tensor.matmul(
                            sc_ps[:, lo:],
                            lhsT=wt_sb[:, 2 * d2 : 2 * d2 + 2, ec * P : (ec + 1) * P],
                            rhs=x8_t[:, 2 * d2 : 2 * d2 + 2, lo:],
                            start=(d2 == 0),
                            stop=(d2 == D2 - 1),
                            perf_mode=mybir.MatmulPerfMode.DoubleRow,
                        )
                    t_t = t_pool.tile([P, BT], F32, tag="tanh")
                    nc.scalar.activation(
                        t_t[:, lo:], sc_ps[:, lo:],
                        mybir.ActivationFunctionType.Tanh,
                        scale=1.0 / W_SCALE,
                    )
                    # exp: init region writes Den directly, accum region to e2
                    if ini_hi > lo:
                        nc.scalar.activation(
                            d_sb[:, ec, lo:ini_hi],
                            t_t[:, lo:ini_hi],
                            mybir.ActivationFunctionType.Exp,
                        )
                    if BT > ini_hi:
                        nc.scalar.activation(
                            e2[:, j, ini_hi:],
                            t_t[:, ini_hi:],
                            mybir.ActivationFunctionType.Exp,
                        )

                    # z_m = prod_ec<ZDET (tanh(s) == 0) over the live range
                    if ec == 0:
                        nc.vector.tensor_single_scalar(
                            out=z_m[:, lo:],
                            in_=t_t[:, lo:],
                            scalar=0.0,
                            op=mybir.AluOpType.is_equal,
                        )
                    elif ec < ZDET_CHUNKS:
                        nc.vector.scalar_tensor_tensor(
                            out=z_m[:, lo:],
                            in0=t_t[:, lo:],
                            scalar=0.0,
                            in1=z_m[:, lo:],
                            op0=mybir.AluOpType.is_equal,
                            op1=mybir.AluOpType.mult,
                        )
                        if ec == ZDET_CHUNKS - 1:
                            nc.vector.tensor_add(
                                scaler_acc[:, lo:], scaler_acc[:, lo:], z_m[:, lo:]
                            )
                            if last_m:
                                nc.vector.tensor_copy(scaler2[:, 0], scaler_acc)
                                nc.vector.tensor_copy(scaler2[:, 1], scaler_acc)

                # paired DVE accumulation over both e-chunks at once
                ecs = slice(2 * ep, 2 * ep + 2)
                if ini_hi > lo:
                    nc.vector.tensor_mul(
                        n_sb[:, ecs, lo:ini_hi],
                        xe_t[:, ecs, lo:ini_hi],
                        d_sb[:, ecs, lo:ini_hi],
                    )
                if BT > ini_hi:
                    p2 = prod_pool.tile([P, 2, BT], FP16, tag="prod")
                    nc.vector.tensor_mul(
                        p2[:, :, ini_hi:], xe_t[:, ecs, ini_hi:], e2[:, :, ini_hi:]
                    )
                    nc.vector.tensor_add(
                        n_sb[:, ecs, ini_hi:], n_sb[:, ecs, ini_hi:], p2[:, :, ini_hi:]
                    )
                    nc.vector.tensor_add(
                        d_sb[:, ecs, ini_hi:], d_sb[:, ecs, ini_hi:], e2[:, :, ini_hi:]
                    )

                if last_m:
                    # scaler is ready after the first pair (zdet chunks 0,1);
                    # finalize pairs as they complete
                    last_chain = bt == NBT - 1 and ep == EP - 1
                    finalize(ep, split=4 if last_chain else 1)

    nc.compile()
    return nc


def plan_shards(x):
    """Assign batch rows to cores so every core sees the same per-modality
    all-zero prefix. Returns (perm [n_cores, Bc] of global row ids,
    skips [M] prefix lengths)."""
    M, B, D = x.shape
    Bc = B // N_CORES
    zero = np.all(x == 0.0, axis=2)  # [M, B] truly-all-zero rows
    zcount = zero.sum(axis=0)
    sig = np.zeros(B, dtype=np.int64)
    for m in range(M):
        sig |= zero[m].astype(np.int64) << m
    # cluster by signature with most-zero rows first; round-robin over cores
    order = np.lexsort((sig, -zcount))
    perm = np.stack([order[c::N_CORES] for c in range(N_CORES)])
    skips = []
    for m in range(M):
        k = Bc
        for c in range(N_CORES):
            nz = np.flatnonzero(~zero[m][perm[c]])
            k = min(k, int(nz[0]) if len(nz) else Bc)
        skips.append(k)
    return perm, skips


def prepare_inputs(x, W, perm):
    """Host-side packing: per-core permuted shard, transposed to [d, b];
    fp8 copy for the matmul path, fp16 copy for the elementwise path."""
    M, B, D = x.shape
    Bc = B // N_CORES
    DC = D // P
    EC = D // P
    wt = np.ascontiguousarray(
        (W.T * W_SCALE)
        .astype(ml_dtypes.float8_e4m3)
        .reshape(DC, P, EC, P)
        .transpose(2, 0, 1, 3)
    )
    x8b = x.astype(ml_dtypes.float8_e4m3)
    xeb = x.astype(np.float16)
    in_maps = []
    for c in range(N_CORES):
        x8c = np.ascontiguousarray(
            x8b[:, perm[c], :].transpose(0, 2, 1)
        ).reshape(M, DC, P, Bc)
        xec = np.ascontiguousarray(
            xeb[:, perm[c], :].transpose(0, 2, 1)
        ).reshape(M, DC, P, Bc)
        in_maps.append({"x8": x8c, "xe": xec, "wt": wt})
    return in_maps


_NC_CACHE = {}


def kernel(x, W, _trace=False, **trace_kwargs):
    x = np.asarray(x)
    W = np.asarray(W)
    M, B, D = x.shape
    Bc = B // N_CORES
    BT = 512 if Bc % 512 == 0 else Bc
    perm, skips = plan_shards(x)
    key = (M, B, D, tuple(skips))
    if key not in _NC_CACHE:
        _NC_CACHE[key] = build_kernel(M, D, D, Bc, BT, skips=skips)
    nc = _NC_CACHE[key]
    in_maps = prepare_inputs(x, W, perm)
    res = run_bass_kernel_spmd(
        nc, in_maps, core_ids=list(range(N_CORES)), trace=_trace, **trace_kwargs
    )
    out = np.empty((B, D), np.float32)
    for c in range(N_CORES):
        out[perm[c], :] = res.results[c]["outT"].T
    if _trace:
        return out, res
    return out


# revision 8
# speedup vs baseline: 1.6074x; 1.2502x over previous
"""Trainium2 kernel for the modality-softmax attention problem.

    scores  = tanh(einsum("mbd,ed->mbe", x, W))
    weights = softmax(scores, axis=0)            # over M modalities
    out     = sum_m x * weights                  # [B, D]
    out    *= (1 + #modalities whose feature-sum is exactly 0)[b]

Sharding: data-parallel over the batch dim — 8 NeuronCores x B/8 rows,
W replicated. Everything on-chip runs transposed ([feature, batch]).

The score matmuls (the FLOP bottleneck: 2*M*B*D^2 = 412 GFLOP) run in
fp8e4 with perf_mode=DoubleRow: each matmul contracts K=256 (two 128-row
sub-chunks packed per PE cell) in the same ~N cycles a bf16 K=128 matmul
takes — a true 2x (measured 216 ns per K=256,N=512 matmul, DR ldweights
fully hidden). W is scaled by 16 before fp8 quantization (its entries
~N(0, 1/D) would land in the subnormal range) and the 1/16 is folded into
the tanh activation's scale. The elementwise path runs in fp16 to keep
quantization error concentrated in the fp8 matmul: host ships two copies
of x — fp8 for the matmul moving operand, fp16 for the elementwise
operand (d-major layout serves both roles since D == E).

Engine budget per core: Tensor ~311 us is the roofline. Every other
engine is kept below it by (a) pair-fusing: scores for two e-chunks
accumulate into one 2-bank PSUM tile so tanh/exp/DVE ops process 1024
columns per instruction, amortizing fixed per-op overhead (measured:
tanh 1114 vs 2x688, 16-bit tensor_tensor 744 vs 2x430); (b) fp16 den
accumulation (reciprocal gets an f32 copy in finalize); (c) pushing the
den-add for middle modalities to the otherwise-idle GpSimd DSP; (d) DMA
rings: wt/x8/out issue on sync, xe on gpsimd, ACT queue kept for
activations only. DRAM layouts are partition-major so every DMA reads
>=2KB contiguous per partition. x tiles are DMA-issued one work-item
ahead; Tile's WAR semaphores start the transfer when the buffer frees.

tanh scores lie in [-1,1], so softmax is computed without max-subtraction
as (sum x*exp(tanh s)) / (sum exp(tanh s)).

Missing-modality rows (all-zero x[m, b, :]) contribute exp(0)=1 to the
softmax denominator and 0 to the numerator, so their matmuls are pure
waste. The host detects them, permutes the batch so each core sees the
same per-modality all-zero prefix, and the kernel is built with those
prefixes statically skipped (shorter matmul N, Den bumped by a constant).
The permutation is undone on the host after the gather. With no zero rows
the plan degenerates to a dense kernel.

Zero-row detection for the rescale: a modality column is all-zero iff its
scores are exactly 0 (fp accumulate of zeros is exact; fp8(0)=0), checked
on the first ZDET_CHUNKS e-chunks of tanh(s); the flags land replicated
across partitions so no cross-partition traffic is needed. The per-pair
finalize (1/Den via fast Newton reciprocal, muls, store) is emitted inside
the last modality's loop so it overlaps the remaining matmuls.
"""

from contextlib import ExitStack

import numpy as np
import ml_dtypes

import concourse.bass as bass
import concourse.bacc as bacc
import concourse.mybir as mybir
import concourse.tile as tile
from concourse.bass_utils import run_bass_kernel_spmd

F32 = mybir.dt.float32
FP16 = mybir.dt.float16
FP8 = mybir.dt.float8e4
P = 128
N_CORES = 8
ZDET_CHUNKS = 2
W_SCALE = 16.0  # pow2: exact in fp; folded into tanh via activation scale
GP_DEN_MODS = (2, 3, 4)  # modalities whose den-add runs on GpSimd


def build_kernel(M, D, E, Bc, BT, skips=None):
    """Build the per-core Bass graph.

    M: modalities, D: contraction dim, E: output feature dim, Bc: per-core
    batch, BT: batch tile (matmul N). skips[m] = per-core all-zero prefix
    length for modality m (columns statically skipped).
    """
    DC = D // P
    D2 = DC // 2  # DoubleRow chunks of K=256
    EC = E // P
    EP = EC // 2  # e-chunk pairs for per-op fusion
    NBT = Bc // BT
    assert D % (2 * P) == 0 and E % (2 * P) == 0 and Bc % BT == 0
    assert ZDET_CHUNKS == 2, "zdet assumed to complete within the first pair"
    skips = list(skips or [0] * M)
    assert len(skips) == M and all(0 <= k <= Bc for k in skips)

    nc = bacc.Bacc()

    x8 = nc.declare_dram_parameter("x8", [M, NBT, P, DC, BT], FP8, isOutput=False)
    xe = nc.declare_dram_parameter("xe", [M, NBT, P, DC, BT], FP16, isOutput=False)
    wt = nc.declare_dram_parameter("wt", [EC, P, DC, P], FP8, isOutput=False)
    outT = nc.declare_dram_parameter("outT", [E, Bc], F32, isOutput=True)

    with tile.TileContext(nc) as tc, ExitStack() as ctx:
        singles = ctx.enter_context(tc.tile_pool(name="singles", bufs=1))
        xe_pool = ctx.enter_context(tc.tile_pool(name="xe", bufs=2))
        x8_pool = ctx.enter_context(tc.tile_pool(name="x8", bufs=2))
        acc_pool = ctx.enter_context(tc.tile_pool(name="acc", bufs=2))
        e_pool = ctx.enter_context(tc.tile_pool(name="e", bufs=3))
        t_pool = ctx.enter_context(tc.tile_pool(name="t", bufs=2))
        prod_pool = ctx.enter_context(tc.tile_pool(name="prod", bufs=2))
        scaler_pool = ctx.enter_context(tc.tile_pool(name="scaler", bufs=2))
        z_pool = ctx.enter_context(tc.tile_pool(name="z", bufs=2))
        out_pool = ctx.enter_context(tc.tile_pool(name="out", bufs=2))
        rec_pool = ctx.enter_context(tc.tile_pool(name="rec", bufs=2))
        den_pool = ctx.enter_context(tc.tile_pool(name="den32", bufs=2))
        sc_psum = ctx.enter_context(tc.tile_pool(name="scps", bufs=3, space="PSUM"))
        wu_psum = ctx.enter_context(tc.tile_pool(name="wups", bufs=1, space="PSUM"))

        dram_pool = ctx.enter_context(tc.tile_pool(name="wudram", bufs=1, space="DRAM"))
        wu_sb = singles.tile([P, 64], mybir.dt.bfloat16)
        nc.vector.memset(wu_sb, 1.0)
        wu_ps = wu_psum.tile([P, 64], F32)
        for i in range(64):
            nc.tensor.matmul(
                wu_ps[:64], lhsT=wu_sb[:, :64], rhs=wu_sb,
                start=(i == 0), stop=(i == 63),
            )
        wu_out = singles.tile([P, 64], F32)
        nc.vector.tensor_copy(wu_out[:64], wu_ps[:64])
        wu_dram = dram_pool.tile([P, 64], F32)
        nc.sync.dma_start(out=wu_dram[:64], in_=wu_out[:64])

        # Work items: one per live (b-tile, modality); x-tile DMAs are issued
        # one item ahead of compute.
        items = []
        for bt in range(NBT):
            for m in range(M):
                sk_m = min(max(skips[m] - bt * BT, 0), BT)
                if sk_m < BT:
                    items.append((bt, m, sk_m))

        tiles = {}

        def issue_loads(it):
            bt, m, sk_m = it
            lo = sk_m
            xe_t = xe_pool.tile([P, DC, BT], FP16)
            x8_t = x8_pool.tile([P, DC, BT], FP8)
            for q in range(0, DC, max(DC // 4, 1)):
                qe = min(q + max(DC // 4, 1), DC)
                nc.sync.dma_start(
                    out=x8_t[:, q:qe, lo:], in_=x8[m, bt, :, q:qe, lo:]
                )
                nc.gpsimd.dma_start(
                    out=xe_t[:, q:qe, lo:], in_=xe[m, bt, :, q:qe, lo:]
                )
            tiles[it] = (xe_t, x8_t)

        # Replicated fp8 weight, resident for the whole kernel; partition-major
        # DRAM layout gives 2KB contiguous per partition per e-chunk. The first
        # x tiles are issued right after the first e-chunk of W.
        wt_sb = singles.tile([P, DC, E], FP8)
        for q in range(0, DC, max(DC // 4, 1)):
            qe = min(q + max(DC // 4, 1), DC)
            nc.sync.dma_start(out=wt_sb[:, q:qe, :P], in_=wt[0, :, q:qe])
        issue_loads(items[0])
        for ec in range(1, EC):
            nc.sync.dma_start(
                out=wt_sb[:, :, ec * P : (ec + 1) * P], in_=wt[ec]
            )

        cur_bt = -1
        scaler_acc = scaler2 = n_sb = d_sb = None
        sk = wm = None
        w_final = 0

        for idx, it in enumerate(items):
            bt, m, _ = it
            if idx + 1 < len(items):
                issue_loads(items[idx + 1])

            if bt != cur_bt:
                cur_bt = bt
                sk = [min(max(skips[mm] - bt * BT, 0), BT) for mm in range(M)]
                wm = [BT] * (M + 1)
                for mm in range(M):
                    wm[mm + 1] = min(wm[mm], sk[mm])
                w_final = wm[M]
                # scaler_acc[e, b] = 1 + #m with all-zero column b (replicated
                # over e). Skipped prefixes are added statically; live ranges
                # come from the z_m flags.
                scaler_acc = scaler_pool.tile([P, BT], F32)
                nc.vector.memset(scaler_acc, 1.0)
                for mm in range(M):
                    if sk[mm] > 0:
                        nc.vector.tensor_scalar_add(
                            scaler_acc[:, : sk[mm]], scaler_acc[:, : sk[mm]], 1.0
                        )
                # duplicated over the pair dim for the pair-wide finalize mul
                scaler2 = scaler_pool.tile([P, 2, BT], F32, tag="s2")
                n_sb = acc_pool.tile([P, EC, BT], FP16, tag="num")
                d_sb = acc_pool.tile([P, EC, BT], FP16, tag="den")

            def finalize(ep, scaler2=scaler2, n_sb=n_sb, d_sb=d_sb,
                         bt=bt, sk=sk, w_final=w_final, split=1):
                ec0 = 2 * ep
                ecs = slice(ec0, ec0 + 2)
                # statically-skipped modalities contribute exp(0)=1 to Den
                if w_final > 0:
                    nc.vector.memset(n_sb[:, ecs, :w_final], 0.0)
                    nc.vector.memset(d_sb[:, ecs, :w_final], 0.0)
                for mm in range(M):
                    if sk[mm] > 0:
                        nc.vector.tensor_scalar_add(
                            d_sb[:, ecs, : sk[mm]], d_sb[:, ecs, : sk[mm]], 1.0
                        )
                d32 = den_pool.tile([P, 2, BT], F32, tag="d32")
                nc.vector.tensor_copy(d32, d_sb[:, ecs, :])
                H = BT // split
                for h in range(split):
                    hs = slice(h * H, (h + 1) * H)
                    r_t = rec_pool.tile([P, 2, BT], F32, tag="recip")
                    # Den in [1, M*e]: no edge cases; 51-ULP approx is plenty
                    nc.vector.reciprocal_approx_fast(
                        out=r_t[:, :, hs], in_=d32[:, :, hs]
                    )
                    # fold the missing-modality scaler into the reciprocal
                    nc.vector.tensor_mul(r_t[:, :, hs], r_t[:, :, hs], scaler2[:, :, hs])
                    o_t = out_pool.tile([P, 2, BT], F32)
                    nc.vector.tensor_mul(o_t[:, :, hs], n_sb[:, ecs, hs], r_t[:, :, hs])
                    nc.sync.dma_start(
                        out=outT[
                            ec0 * P : (ec0 + 2) * P,
                            bt * BT + h * H : bt * BT + (h + 1) * H,
                        ].rearrange("(c p) b -> p c b", c=2),
                        in_=o_t[:, :, hs],
                    )

            lo = sk[m]  # live range [lo, BT)
            ini_hi = wm[m]  # init range [lo, ini_hi), accum [ini_hi, BT)
            xe_t, x8_t = tiles.pop(it)
            z_m = z_pool.tile([P, BT], F32)
            last_m = m == M - 1

            for ep in range(EP):
                sc_ps = sc_psum.tile([P, 2, BT], F32)
                for j in range(2):
                    ec = 2 * ep + j
                    for d2 in range(D2):
                        nc.tensor.matmul(
                            sc_ps[:, j, lo:],
                            lhsT=wt_sb[:, 2 * d2 : 2 * d2 + 2, ec * P : (ec + 1) * P],
                            rhs=x8_t[:, 2 * d2 : 2 * d2 + 2, lo:],
                            start=(d2 == 0),
                            stop=(d2 == D2 - 1),
                            perf_mode=mybir.MatmulPerfMode.DoubleRow,
                        )
                # pair-fused activations: tanh over both psum banks at once
                t2 = t_pool.tile([P, 2, BT], F32, tag="tanh")
                nc.scalar.activation(
                    t2[:, :, lo:], sc_ps[:, :, lo:],
                    mybir.ActivationFunctionType.Tanh,
                    scale=1.0 / W_SCALE,
                )
                ecs = slice(2 * ep, 2 * ep + 2)
                e2 = None
                if ini_hi > lo:
                    nc.scalar.activation(
                        d_sb[:, ecs, lo:ini_hi],
                        t2[:, :, lo:ini_hi],
                        mybir.ActivationFunctionType.Exp,
                    )
                if BT > ini_hi:
                    e2 = e_pool.tile([P, 2, BT], FP16, tag="exp")
                    nc.scalar.activation(
                        e2[:, :, ini_hi:],
                        t2[:, :, ini_hi:],
                        mybir.ActivationFunctionType.Exp,
                    )

                # z_m = prod_ec<ZDET (tanh(s) == 0) over the live range
                if ep == 0:
                    nc.vector.tensor_single_scalar(
                        out=z_m[:, lo:],
                        in_=t2[:, 0, lo:],
                        scalar=0.0,
                        op=mybir.AluOpType.is_equal,
                    )
                    nc.vector.scalar_tensor_tensor(
                        out=z_m[:, lo:],
                        in0=t2[:, 1, lo:],
                        scalar=0.0,
                        in1=z_m[:, lo:],
                        op0=mybir.AluOpType.is_equal,
                        op1=mybir.AluOpType.mult,
                    )
                    nc.vector.tensor_add(
                        scaler_acc[:, lo:], scaler_acc[:, lo:], z_m[:, lo:]
                    )
                    if last_m:
                        nc.vector.tensor_copy(scaler2[:, 0], scaler_acc)
                        nc.vector.tensor_copy(scaler2[:, 1], scaler_acc)

                # paired DVE accumulation over both e-chunks at once
                if ini_hi > lo:
                    nc.vector.tensor_mul(
                        n_sb[:, ecs, lo:ini_hi],
                        xe_t[:, ecs, lo:ini_hi],
                        d_sb[:, ecs, lo:ini_hi],
                    )
                if BT > ini_hi:
                    p2 = prod_pool.tile([P, 2, BT], FP16, tag="prod")
                    nc.vector.tensor_mul(
                        p2[:, :, ini_hi:], xe_t[:, ecs, ini_hi:], e2[:, :, ini_hi:]
                    )
                    nc.vector.tensor_add(
                        n_sb[:, ecs, ini_hi:], n_sb[:, ecs, ini_hi:], p2[:, :, ini_hi:]
                    )
                    den_eng = nc.gpsimd if m in GP_DEN_MODS else nc.vector
                    den_eng.tensor_add(
                        d_sb[:, ecs, ini_hi:], d_sb[:, ecs, ini_hi:], e2[:, :, ini_hi:]
                    )

                if last_m:
                    # scaler is ready after the first pair (zdet chunks 0,1);
                    # finalize pairs as they complete
                    last_chain = bt == NBT - 1 and ep == EP - 1
                    finalize(ep, split=4 if last_chain else 1)

    nc.compile()
    return nc


def plan_shards(x):
    """Assign batch rows to cores so every core sees the same per-modality
    all-zero prefix. Returns (perm [n_cores, Bc] of global row ids,
    skips [M] prefix lengths)."""
    M, B, D = x.shape
    Bc = B // N_CORES
    zero = np.all(x == 0.0, axis=2)  # [M, B] truly-all-zero rows
    zcount = zero.sum(axis=0)
    sig = np.zeros(B, dtype=np.int64)
    for m in range(M):
        sig |= zero[m].astype(np.int64) << m
    # cluster by signature with most-zero rows first; round-robin over cores
    order = np.lexsort((sig, -zcount))
    perm = np.stack([order[c::N_CORES] for c in range(N_CORES)])
    skips = []
    for m in range(M):
        k = Bc
        for c in range(N_CORES):
            nz = np.flatnonzero(~zero[m][perm[c]])
            k = min(k, int(nz[0]) if len(nz) else Bc)
        skips.append(k)
    return perm, skips


def prepare_inputs(x, W, perm, BT):
    """Host-side packing: per-core permuted shard, partition-major layouts;
    fp8 copy for the matmul path, fp16 copy for the elementwise path."""
    M, B, D = x.shape
    Bc = B // N_CORES
    DC = D // P
    EC = D // P
    NBT = Bc // BT
    # wt[ec, p, dc, j] = (W.T * S)[dc*P + p, ec*P + j]
    wt = np.ascontiguousarray(
        (W.T * W_SCALE)
        .astype(ml_dtypes.float8_e4m3)
        .reshape(DC, P, EC, P)
        .transpose(2, 1, 0, 3)
    )
    x8b = x.astype(ml_dtypes.float8_e4m3)
    xeb = x.astype(np.float16)
    in_maps = []
    for c in range(N_CORES):
        # [M, NBT, P, DC, BT] with value (m, bt, p, dc, b) =
        #   x[m, perm[bt*BT+b], dc*P+p]
        x8c = np.ascontiguousarray(
            x8b[:, perm[c], :].reshape(M, NBT, BT, DC, P).transpose(0, 1, 4, 3, 2)
        )
        xec = np.ascontiguousarray(
            xeb[:, perm[c], :].reshape(M, NBT, BT, DC, P).transpose(0, 1, 4, 3, 2)
        )
        in_maps.append({"x8": x8c, "xe": xec, "wt": wt})
    return in_maps


_NC_CACHE = {}


def kernel(x, W, _trace=False, **trace_kwargs):
    x = np.asarray(x)
    W = np.asarray(W)
    M, B, D = x.shape
    Bc = B // N_CORES
    BT = 512 if Bc % 512 == 0 else Bc
    perm, skips = plan_shards(x)
    key = (M, B, D, tuple(skips))
    if key not in _NC_CACHE:
        _NC_CACHE[key] = build_kernel(M, D, D, Bc, BT, skips=skips)
    nc = _NC_CACHE[key]
    in_maps = prepare_inputs(x, W, perm, BT)
    res = run_bass_kernel_spmd(
        nc, in_maps, core_ids=list(range(N_CORES)), trace=_trace, **trace_kwargs
    )
    out = np.empty((B, D), np.float32)
    for c in range(N_CORES):
        out[perm[c], :] = res.results[c]["outT"].T
    if _trace:
        return out, res
    return out
